# revision 26
# baseline (speedup 1.0000x reference)
"""TRN2 Bass kernel for nn_Aligner (dense_transformer).

Reference computation (per batch b):
    ex  = ix_b @ W.T + b         [L, D]
    eo  = io_b @ W.T + b         [L, D]
    s   = ex @ eo.T              [L, L]
    a   = softmax(s, axis=-1)
    out = a @ io_b               [L, D]

Device algorithm (algebraic restructure; softmax is shift-invariant):
    s[l, m] = ix_l @ M @ io_m + c_m + (const per row l, dropped in softmax)
      where M = W.T @ W (symmetric, host-precomputed),
            c = io @ (W.T b)    (host-precomputed)
    tT[e, l]  = sum_d M[d, e] ixT[d, l]            (step A)
    sT[m, l]  = sum_e ioT[e, m] tT[e, l]           (step B)
    E         = exp(sT + c[m])   (no max-subtract; fp32 range is ample)
    sums[l]   = sum_m E[m, l]    (ones-matmul over partitions)
    out[l, d] = (sum_m E[m, l] io[m, d]) / sums[l] (step C + normalize)

This removes the two dense projections (1.5x fewer FLOPs) and all matmuls
run in fp32r (full PE speed, ~1.2e-4 rounding vs 4x-slower fp32).

Layout: the TensorE contracts along partitions, so ix must arrive with d on
partitions and io in both [e, m] and [m, d] layouts. The host ships ixT/ioT
(a transpose done while sharding) so the device spends zero cycles on
transposes — it is a pure matmul + softmax pipeline.

Sharding: pure data-parallel over batch. 32 batches / 8 cores = 4 per core.
M and ones are replicated; no collectives.
"""
import numpy as np
from contextlib import ExitStack

import concourse.bacc as bacc
import concourse.mybir as mybir
from concourse import tile

B, L, D = 32, 512, 1024
NCORES = 8
BL = B // NCORES          # batches per core
P = 128
DC = D // P               # 8 contraction chunks over d/e
LC = L // P               # 4 chunks over l
MC = L // P               # 4 chunks over m
HD = D // 2               # 512: fp32 PSUM free-dim limit per matmul
F32 = mybir.dt.float32
F32R = mybir.dt.float32r
EXP = mybir.ActivationFunctionType.Exp

_CACHE = {}


def _build_program():
    nc = bacc.Bacc("TRN2", target_bir_lowering=False, debug=False,
                   num_devices=NCORES)
    ixt_d = nc.dram_tensor("ixt", [BL, D, L], F32, kind="ExternalInput")
    iot_d = nc.dram_tensor("iot", [BL, D, L], F32, kind="ExternalInput")
    io_d = nc.dram_tensor("io", [BL, L, D], F32, kind="ExternalInput")
    mw_d = nc.dram_tensor("mw", [D, D], F32, kind="ExternalInput")
    cv_d = nc.dram_tensor("cv", [BL, L], F32, kind="ExternalInput")
    out_d = nc.dram_tensor("out", [BL, L, D], F32, kind="ExternalOutput")

    ones_dram = nc.inline_tensor(np.ones((P, 2), dtype=np.float32), name="ones_c")

    with tile.TileContext(nc) as tc, ExitStack() as ctx:
        const = ctx.enter_context(tc.tile_pool(name="const", bufs=1))
        ixt_pool = ctx.enter_context(tc.tile_pool(name="ixt", bufs=2))
        iot_pool = ctx.enter_context(tc.tile_pool(name="iot", bufs=2))
        ion_pool = ctx.enter_context(tc.tile_pool(name="ion", bufs=2))
        t_pool = ctx.enter_context(tc.tile_pool(name="tp", bufs=1))
        e_pool = ctx.enter_context(tc.tile_pool(name="ep", bufs=1))
        out_pool = ctx.enter_context(tc.tile_pool(name="op", bufs=4))
        small = ctx.enter_context(tc.tile_pool(name="small", bufs=2))

        mm_psum = ctx.enter_context(tc.tile_pool(name="mmp", bufs=5, space="PSUM"))
        c_psum = ctx.enter_context(tc.tile_pool(name="cp", bufs=3, space="PSUM"))

        def load_batch(b):
            cvt = small.tile([P, MC], F32, tag="cvt")
            nc.sync.dma_start(cvt[:], cv_d[b].rearrange("(mc p) -> p mc", p=P))
            ixT = ixt_pool.tile([P, DC * L], F32R, tag="ixT")
            for dc in range(DC):
                nc.sync.dma_start(ixT[:, dc * L:(dc + 1) * L],
                                  ixt_d[b, dc * P:(dc + 1) * P, :].bitcast(F32R))
            ioT = iot_pool.tile([P, DC * L], F32R, tag="ioT")
            for dc in range(DC):
                nc.sync.dma_start(ioT[:, dc * L:(dc + 1) * L],
                                  iot_d[b, dc * P:(dc + 1) * P, :].bitcast(F32R))
            ion = ion_pool.tile([P, LC * D], F32R, tag="ion")
            for lc in range(LC):
                nc.sync.dma_start(ion[:, lc * D:(lc + 1) * D],
                                  io_d[b, lc * P:(lc + 1) * P, :].bitcast(F32R))
            return ixT, ioT, ion, cvt

        # Startup-latency ordering: step A's first matmul needs only
        # ixT[dc=0] and M[dc=0,ec=0], so those DMAs go absolutely first
        # (mw[0] split so the 64KB first block lands immediately); the ones
        # vector + exp-table warm-up aren't needed until the first softmax
        # (~t=30us) and ride behind the cold stream.
        ones2 = const.tile([P, 2], F32R)
        nc.sync.dma_start(ones2[:], ones_dram.ap().bitcast(F32R))
        warm = small.tile([P, 2], F32, tag="warm")
        nc.scalar.activation(warm[:], ones2[:], EXP)
        # PE p-state warm-up: keep TensorE busy on resident data during the
        # DMA-bound cold fill so the ~3us ramp elapses before real matmuls.
        warm_ps = mm_psum.tile([P, 2], F32, tag="mm", name="warmps")
        for i in range(60):
            nc.tensor.matmul(warm_ps[:2, :], ones2[:], ones2[:],
                             start=(i == 0), stop=(i == 59))
        # M chunks side by side: mw_sb[:, dc*D:(dc+1)*D] = M[dc*128:(dc+1)*128, :]
        mw_sb = const.tile([P, DC * D], F32R)
        ixT0 = ixt_pool.tile([P, DC * L], F32R, tag="ixT")
        cvt0 = small.tile([P, MC], F32, tag="cvt")
        nc.sync.dma_start(ixT0[:, 0:L], ixt_d[0, 0:P, :].bitcast(F32R))
        nc.sync.dma_start(mw_sb[:, 0:D], mw_d[0:P, :].bitcast(F32R))
        for dc in range(1, DC):
            nc.sync.dma_start(ixT0[:, dc * L:(dc + 1) * L],
                              ixt_d[0, dc * P:(dc + 1) * P, :].bitcast(F32R))
            nc.sync.dma_start(mw_sb[:, dc * D:(dc + 1) * D],
                              mw_d[dc * P:(dc + 1) * P, :].bitcast(F32R))
            if dc == 1:
                nc.sync.dma_start(cvt0[:],
                                  cv_d[0].rearrange("(mc p) -> p mc", p=P))
        ioT0 = iot_pool.tile([P, DC * L], F32R, tag="ioT")
        for dc in range(DC):
            nc.sync.dma_start(ioT0[:, dc * L:(dc + 1) * L],
                              iot_d[0, dc * P:(dc + 1) * P, :].bitcast(F32R))
        ion0 = ion_pool.tile([P, LC * D], F32R, tag="ion")
        for lc in range(LC):
            nc.sync.dma_start(ion0[:, lc * D:(lc + 1) * D],
                              io_d[0, lc * P:(lc + 1) * P, :].bitcast(F32R))
        batch0 = (ixT0, ioT0, ion0, cvt0)

        for b in range(BL):
            ixT, ioT, ion, cvt = batch0 if b == 0 else load_batch(b)

            # ---- step A: tT[e, l] ----
            # dc-outer, single pass with 8 concurrent PSUM groups (5 from the
            # mm pool + 3 borrowed from the then-idle C pool): each arriving
            # (ixT[dc], M[dc]) chunk pair feeds 8 matmuls, halving the cold
            # HBM demand per compute vs a 4-group half-pass.
            tT = t_pool.tile([P, DC * L], F32R, tag="tT")
            pas = [mm_psum.tile([P, L], F32, tag="mm", name=f"pa{j}")
                   for j in range(5)]
            pas += [c_psum.tile([P, L], F32, tag="pc", name=f"pa{j}")
                    for j in range(5, DC)]
            for dc in range(DC):
                for ec, pa in enumerate(pas):
                    nc.tensor.matmul(
                        pa[:],
                        mw_sb[:, dc * D + ec * P: dc * D + (ec + 1) * P],
                        ixT[:, dc * L:(dc + 1) * L],
                        start=(dc == 0), stop=(dc == DC - 1))
            for ec, pa in enumerate(pas):
                nc.vector.tensor_copy(tT[:, ec * L:(ec + 1) * L], pa[:])

            # ---- step B + exp: E[m, l] = exp(sT + c_m) ----
            esb = e_pool.tile([P, MC * L], F32R, tag="esb")
            for mc in range(MC):
                pb = mm_psum.tile([P, L], F32, tag="mm")
                for ec in range(DC):
                    nc.tensor.matmul(
                        pb[:],
                        ioT[:, ec * L + mc * P: ec * L + (mc + 1) * P],
                        tT[:, ec * L:(ec + 1) * L],
                        start=(ec == 0), stop=(ec == DC - 1))
                nc.scalar.activation(esb[:, mc * L:(mc + 1) * L], pb[:],
                                     EXP, bias=cvt[:, mc:mc + 1])

            # ---- softmax denominators ----
            sums = small.tile([P, LC], F32, tag="sums")
            for lc in range(LC):
                pss = mm_psum.tile([P, 2], F32, tag="mm", name="pss")
                for mc in range(MC):
                    nc.tensor.matmul(
                        pss[:],
                        esb[:, mc * L + lc * P: mc * L + (lc + 1) * P],
                        ones2[:],
                        start=(mc == 0), stop=(mc == MC - 1))
                nc.vector.tensor_copy(sums[:, lc:lc + 1], pss[:, 0:1])

            rec = small.tile([P, LC], F32, tag="rec")
            nc.vector.reciprocal(rec[:], sums[:])

            # ---- step C + normalize ----
            for lc in range(LC):
                pc0 = c_psum.tile([P, HD], F32, tag="pc")
                pc1 = c_psum.tile([P, HD], F32, tag="pc")
                # all pc0 matmuls first: its normalize+DMA then overlap
                # pc1's matmuls, shortening the per-lc (and kernel) tail
                for mc in range(MC):
                    lhs = esb[:, mc * L + lc * P: mc * L + (lc + 1) * P]
                    nc.tensor.matmul(pc0[:], lhs, ion[:, mc * D: mc * D + HD],
                                     start=(mc == 0), stop=(mc == MC - 1))
                for mc in range(MC):
                    lhs = esb[:, mc * L + lc * P: mc * L + (lc + 1) * P]
                    nc.tensor.matmul(pc1[:], lhs,
                                     ion[:, mc * D + HD:(mc + 1) * D],
                                     start=(mc == 0), stop=(mc == MC - 1))
                outt = out_pool.tile([P, D], F32, tag="outt")
                nc.vector.tensor_scalar_mul(outt[:, 0:HD], pc0[:],
                                            rec[:, lc:lc + 1])
                nc.sync.dma_start(out_d[b, lc * P:(lc + 1) * P, 0:HD],
                                  outt[:, 0:HD])
                nc.vector.tensor_scalar_mul(outt[:, HD:D], pc1[:],
                                            rec[:, lc:lc + 1])
                nc.sync.dma_start(out_d[b, lc * P:(lc + 1) * P, HD:D],
                                  outt[:, HD:D])

    nc.compile()
    return nc


def _host_prep(ix, iother, W, b):
    """Shard + layout prep: M = W.T W, c = io @ (W.T b), ixT/ioT transposes."""
    M = np.ascontiguousarray(W.T) @ W                       # [D, D] fp32 sgemm
    u = W.T @ b                                             # [D]
    c = iother.reshape(-1, D) @ u                           # [B*L]
    c = c.reshape(B, L).astype(np.float32)
    ixt = np.ascontiguousarray(ix.transpose(0, 2, 1))       # [B, D, L]
    iot = np.ascontiguousarray(iother.transpose(0, 2, 1))   # [B, D, L]
    in_maps = []
    for core in range(NCORES):
        sl = slice(core * BL, (core + 1) * BL)
        in_maps.append({
            "ixt": ixt[sl],
            "iot": iot[sl],
            "io": np.ascontiguousarray(iother[sl]),
            "mw": M,
            "cv": np.ascontiguousarray(c[sl]),
        })
    return in_maps


def _get_nc():
    if "nc" not in _CACHE:
        _CACHE["nc"] = _build_program()
    return _CACHE["nc"]


def _get_runner():
    """Compile once; return (fn, in_names, out_names, out_shapes).

    Mirrors bass2jax.run_bass_via_pjrt's multi-core path but caches the
    jitted executable so repeated kernel() calls skip recompilation.
    """
    if "runner" in _CACHE:
        return _CACHE["runner"]
    import jax
    from jax.sharding import Mesh, PartitionSpec
    from jax.experimental.shard_map import shard_map
    from concourse import bass2jax
    from concourse import mybir as mb

    nc = _get_nc()
    bass2jax.install_neuronx_cc_hook()

    partition_name = (nc.partition_id_tensor.name
                      if nc.partition_id_tensor else None)
    in_names, out_names, out_avals, zero_shapes = [], [], [], []
    for alloc in nc.m.functions[0].allocations:
        if not isinstance(alloc, mb.MemoryLocationSet):
            continue
        name = alloc.memorylocations[0].name
        if alloc.kind == "ExternalInput":
            if name != partition_name:
                in_names.append(name)
        elif alloc.kind == "ExternalOutput":
            out_names.append(name)
            shape = tuple(alloc.tensor_shape)
            dtype = mb.dt.np(alloc.dtype)
            out_avals.append(jax.core.ShapedArray(shape, dtype))
            zero_shapes.append((shape, dtype))
    n_params = len(in_names)
    all_in_names = in_names + out_names
    if partition_name is not None:
        all_in_names = all_in_names + [partition_name]

    def _body(*args):
        operands = list(args)
        if partition_name is not None:
            operands.append(bass2jax.partition_id_tensor())
        outs = bass2jax._bass_exec_p.bind(
            *operands,
            out_avals=tuple(out_avals),
            in_names=tuple(all_in_names),
            out_names=tuple(out_names),
            lowering_input_output_aliases=(),
            sim_require_finite=True,
            sim_require_nnan=True,
            nc=nc,
        )
        return tuple(outs)

    devices = jax.devices()[:NCORES]
    mesh = Mesh(np.asarray(devices), ("core",))
    in_specs = (PartitionSpec("core"),) * (n_params + len(out_names))
    out_specs = (PartitionSpec("core"),) * len(out_names)
    donate = tuple(range(n_params, n_params + len(out_names)))
    fn = jax.jit(
        shard_map(_body, mesh=mesh, in_specs=in_specs, out_specs=out_specs,
                  check_rep=False),
        donate_argnums=donate, keep_unused=True)
    _CACHE["runner"] = (fn, in_names, out_names, zero_shapes)
    return _CACHE["runner"]


def _run(in_maps):
    fn, in_names, out_names, zero_shapes = _get_runner()
    concat_in = [
        np.concatenate([in_maps[c][name] for c in range(NCORES)], axis=0)
        for name in in_names
    ]
    zeros = [np.zeros((NCORES * s[0], *s[1:]), dt) for s, dt in zero_shapes]
    out_arrs = fn(*concat_in, *zeros)
    return {name: np.asarray(out_arrs[i]) for i, name in enumerate(out_names)}


def kernel(ix, iother, W, b):
    ix = np.asarray(ix, dtype=np.float32)
    iother = np.asarray(iother, dtype=np.float32)
    W = np.asarray(W, dtype=np.float32)
    b = np.asarray(b, dtype=np.float32)
    in_maps = _host_prep(ix, iother, W, b)
    # One retry: the device occasionally reports a transient
    # NRT_EXEC_UNIT_UNRECOVERABLE under rapid back-to-back runs.
    try:
        outs = _run(in_maps)
    except Exception:
        import time
        time.sleep(2.0)
        outs = _run(in_maps)
    return outs["out"].astype(np.float32)
